# revision 30
# baseline (speedup 1.0000x reference)
"""Trainium2 Bass kernel for nn_CA_41936060678207 (sparse_attention).

Reference computation:
    Q = MLP_q(x), K = MLP_k(x)            # two-layer ReLU MLPs, per token
    S = Q @ K^T                           # [B, M, M]
    out = softmax(TEMP * S / rowmax(S))   # row-max-normalized softmax

Sharding: data-parallel over batch B=4 across 8 cores (2 cores per batch),
sequence-parallel over query rows within a batch (2048 rows per core).
Each core computes K for the full 4096-token sequence and Q for its own
2048 query rows, then its [2048, 4096] slab of the output.

Layouts: host pre-transposes x to [D, M] (fp16) so both MLPs run with the
contraction dim on partitions and produce Q^T/K^T directly; no on-device
transposes are needed. Scores accumulate in fp32 PSUM; a fused custom-DVE
op (TENSOR_MASK_REDUCE) drains PSUM->SBUF while computing the row max;
exp runs on ScalarE with a per-row scale and a fused row-sum; the final
normalize runs on DVE in fp16 (4x mode); the fp16 output slab is upcast
to fp32 on the host.
"""

import numpy as np

import concourse.mybir as mybir
import concourse.tile as tile
from concourse import bacc, bass_utils
from concourse.dve_ops import TENSOR_MASK_REDUCE

# Problem constants (hardcoded per harness contract)
B, M, D_IN, D_HID, D_OUT = 4, 4096, 256, 512, 256
TEMP = 10.0
NCORES = 8
CORES_PER_BATCH = NCORES // B          # 2
MQ = M // CORES_PER_BATCH              # 2048 query rows per core
P = 128                                # partitions
F16 = mybir.dt.float16
F32 = mybir.dt.float32

NEG_INIT = -3.0e38                     # max-reduce init (< any fp32 score)
TB = 1024                              # MLP token block per activation


def _emit_mlp_block(nc, pool_ps, pool_h, w1, w2, xt, yt, t0, pfx):
    """One TB-token block of a two-layer MLP in transposed layout."""
    if True:
        ht = [
            pool_h.tile([P, TB], F16, tag=f"h{m}", name=f"{pfx}h{m}")
            for m in range(D_HID // P)
        ]
        # layer 1: h^T[m] = relu(sum_c W1[c, m*128:+128]^T @ x^T[c])
        for m in range(D_HID // P):
            ph = pool_ps.tile([P, TB], F32, tag="mlp", name="ph")
            for c in range(2):
                for s0 in range(0, TB, 512):
                    nc.tensor.matmul(
                        ph[:, s0 : s0 + 512],
                        w1[c][:, m * P : (m + 1) * P],
                        xt[c][:, t0 + s0 : t0 + s0 + 512],
                        start=(c == 0),
                        stop=(c == 1),
                    )
            nc.scalar.activation(
                out=ht[m],
                in_=ph,
                func=mybir.ActivationFunctionType.Relu,
                bias=0.0,
                scale=1.0,
            )
        # layer 2: y^T[o] = sum_c W2[c, o*128:+128]^T @ h^T[c]
        for o in range(D_OUT // P):
            po = pool_ps.tile([P, TB], F32, tag="mlp", name="po")
            for c in range(4):
                for s0 in range(0, TB, 512):
                    nc.tensor.matmul(
                        po[:, s0 : s0 + 512],
                        w2[c][:, o * P : (o + 1) * P],
                        ht[c][:, s0 : s0 + 512],
                        start=(c == 0),
                        stop=(c == 3),
                    )
            nc.scalar.activation(
                out=yt[o][t0 // TB],
                in_=po,
                func=mybir.ActivationFunctionType.Copy,
                bias=0.0,
                scale=1.0,
            )


def build_nc():
    nc = bacc.Bacc("TRN2", target_bir_lowering=False, debug=False)

    xt_d = nc.dram_tensor("xt", [D_IN, M], F16, kind="ExternalInput")
    xqt_d = nc.dram_tensor("xqt", [D_IN, MQ], F16, kind="ExternalInput")
    w1k_d = nc.dram_tensor("w1k", [D_IN, D_HID], F16, kind="ExternalInput")
    w2k_d = nc.dram_tensor("w2k", [D_HID, D_OUT], F16, kind="ExternalInput")
    w1q_d = nc.dram_tensor("w1q", [D_IN, D_HID], F16, kind="ExternalInput")
    w2q_d = nc.dram_tensor("w2q", [D_HID, D_OUT], F16, kind="ExternalInput")
    out_d = nc.dram_tensor("out", [MQ, M], F16, kind="ExternalOutput")

    with tile.TileContext(nc) as tc:
        with tc.tile_pool(name="const", bufs=1) as cp, \
             tc.tile_pool(name="hpool", bufs=3) as hp, \
             tc.tile_pool(name="spool", bufs=5) as sp, \
             tc.tile_pool(name="epool", bufs=3) as ep, \
             tc.tile_pool(name="small", bufs=22) as smp:

            # ---- load inputs (weights first; x in column chunks) ----
            def load_w(dram, rows, cols, name):
                tiles = []
                for c in range(rows // P):
                    t = cp.tile([P, cols], F16, tag=f"{name}{c}", name=f"{name}{c}")
                    nc.sync.dma_start(out=t, in_=dram.ap()[c * P : (c + 1) * P, :])
                    tiles.append(t)
                return tiles

            w1q = load_w(w1q_d, D_IN, D_HID, "w1q")
            w2q = load_w(w2q_d, D_HID, D_OUT, "w2q")
            w1k = load_w(w1k_d, D_IN, D_HID, "w1k")
            w2k = load_w(w2k_d, D_HID, D_OUT, "w2k")

            xq = [cp.tile([P, MQ], F16, tag=f"xq{c}", name=f"xq{c}") for c in range(2)]
            for t0 in range(0, MQ, TB):
                for c in range(2):
                    nc.scalar.dma_start(
                        out=xq[c][:, t0 : t0 + TB],
                        in_=xqt_d.ap()[c * P : (c + 1) * P, t0 : t0 + TB],
                    )
            xk = [cp.tile([P, M], F16, tag=f"xk{c}", name=f"xk{c}") for c in range(2)]
            for t0 in range(0, M, TB):
                for c in range(2):
                    nc.scalar.dma_start(
                        out=xk[c][:, t0 : t0 + TB],
                        in_=xt_d.ap()[c * P : (c + 1) * P, t0 : t0 + TB],
                    )

            # constants
            c3_q = smp.tile([P, 1], F32, tag="c3q", name="c3q")
            nc.vector.memset(c3_q, float(TB))
            bias_mT = smp.tile([P, 1], F32, tag="bT", name="bT")
            nc.vector.memset(bias_mT, -TEMP)

            # K^T / Q^T in TB-wide segment tiles so downstream S matmuls
            # can start as soon as individual segments are written
            kt = [
                [cp.tile([P, TB], F16, tag=f"kt{c}s{s}", name=f"kt{c}s{s}")
                 for s in range(M // TB)]
                for c in range(2)
            ]
            qt = [
                [cp.tile([P, TB], F16, tag=f"qt{c}s{s}", name=f"qt{c}s{s}")
                 for s in range(MQ // TB)]
                for c in range(2)
            ]

            # ---- MLPs + S phase share one PSUM pool so the scheduler can
            # overlap K-MLP with early S matmuls (segment-level deps).
            # Emission order interleaves K/Q blocks so the first S quarter
            # (kt seg0 + qt seg0) is ready as early as possible. ----
            with tc.tile_pool(name="psum", bufs=2, space="PSUM") as psum:
                for t0 in range(0, MQ, TB):
                    _emit_mlp_block(nc, psum, hp, w1q, w2q, xq, qt, t0, "q")
                for t0 in range(0, M, TB):
                    _emit_mlp_block(nc, psum, hp, w1k, w2k, xk, kt, t0, "k")

                NQ = M // TB                       # 4 quarter-row blocks
                for i in range(MQ // P):           # 16 query tiles
                    s32 = sp.tile([P, M], F32, tag="s32", name="s32")
                    mx4 = smp.tile([P, NQ], F32, tag="mx4", name="mx4")
                    for q in range(NQ):
                        ps = psum.tile(
                            [P, TB], F32,
                            tag="ps" if q % 2 == 0 else "mlp", name="ps",
                        )
                        for c in range(2):         # weights outer: fewer loads
                            for n0 in range(0, TB, 512):
                                nc.tensor.matmul(
                                    ps[:, n0 : n0 + 512],
                                    qt[c][i * P // TB][:, i * P % TB : i * P % TB + P],
                                    kt[c][q][:, n0 : n0 + 512],
                                    start=(c == 0),
                                    stop=(c == 1),
                                )
                        # fused PSUM->SBUF copy + row-max
                        nc.vector._custom_dve(
                            TENSOR_MASK_REDUCE,
                            out=s32[:, q * TB : (q + 1) * TB],
                            in0=ps,
                            in1=c3_q,
                            s0=0.0,
                            s1=NEG_INIT,
                            imm2=1.0,
                            accum_out=mx4[:, q : q + 1],
                        )
                    rowmax = smp.tile([P, 1], F32, tag="rowmax", name="rowmax")
                    nc.vector.reduce_max(rowmax, mx4, axis=mybir.AxisListType.X)
                    rmax = smp.tile([P, 1], F32, tag="rmax", name="rmax")
                    nc.vector.reciprocal(rmax, rowmax)
                    scl = smp.tile([P, 1], F32, tag="scl", name="scl")
                    # tiny op on idle GpSimd keeps the DVE FIFO clear
                    nc.gpsimd.tensor_scalar_mul(scl, rmax, TEMP)
                    rowsum = smp.tile([P, 1], F32, tag="rowsum", name="rowsum")
                    # e = exp(TEMP/rowmax * S - TEMP) -> fp16; rowsum fused fp32
                    e16 = ep.tile([P, M], F16, tag="e16", name="e16")
                    nc.scalar.activation(
                        out=e16,
                        in_=s32,
                        func=mybir.ActivationFunctionType.Exp,
                        bias=bias_mT,
                        scale=scl,
                        accum_out=rowsum,
                    )
                    rsum = smp.tile([P, 1], F32, tag="rsum", name="rsum")
                    nc.vector.reciprocal(rsum, rowsum)
                    # normalize on DVE in fp16 (4x mode), in place
                    nc.vector.tensor_scalar_mul(e16, e16, rsum)
                    nc.sync.dma_start(
                        out=out_d.ap()[i * P : (i + 1) * P, :], in_=e16
                    )

    nc.compile()
    return nc


_NC_CACHE = None


def _get_nc():
    global _NC_CACHE
    if _NC_CACHE is None:
        _NC_CACHE = build_nc()
    return _NC_CACHE


def make_in_maps(x, Wq1, Wq2, Wk1, Wk2):
    """Per-core input dicts (host-side sharding + layout prep)."""
    x = np.asarray(x, dtype=np.float32)
    w1q = np.asarray(Wq1, dtype=np.float16)
    w2q = np.asarray(Wq2, dtype=np.float16)
    w1k = np.asarray(Wk1, dtype=np.float16)
    w2k = np.asarray(Wk2, dtype=np.float16)
    xt16 = [np.ascontiguousarray(x[b].T).astype(np.float16) for b in range(B)]
    in_maps = []
    for c in range(NCORES):
        b, h = divmod(c, CORES_PER_BATCH)
        in_maps.append(
            {
                "xt": xt16[b],
                "xqt": np.ascontiguousarray(xt16[b][:, h * MQ : (h + 1) * MQ]),
                "w1k": w1k,
                "w2k": w2k,
                "w1q": w1q,
                "w2q": w2q,
            }
        )
    return in_maps


def kernel(x, Wq1, bq1, Wq2, bq2, Wk1, bk1, Wk2, bk2):
    for b_ in (bq1, bq2, bk1, bk2):
        assert not np.any(np.asarray(b_)), "kernel assumes zero MLP biases"

    in_maps = make_in_maps(x, Wq1, Wq2, Wk1, Wk2)
    nc = _get_nc()
    res = bass_utils.run_bass_kernel_spmd(nc, in_maps, core_ids=list(range(NCORES)))

    out = np.empty((B, M, M), dtype=np.float32)
    for c, r in enumerate(res.results):
        b, h = divmod(c, CORES_PER_BATCH)
        out[b, h * MQ : (h + 1) * MQ, :] = r["out"].astype(np.float32)
    return out


# revision 31
# speedup vs baseline: 1.0122x; 1.0122x over previous
"""Trainium2 Bass kernel for nn_CA_41936060678207 (sparse_attention).

Reference computation:
    Q = MLP_q(x), K = MLP_k(x)            # two-layer ReLU MLPs, per token
    S = Q @ K^T                           # [B, M, M]
    out = softmax(TEMP * S / rowmax(S))   # row-max-normalized softmax

Sharding: data-parallel over batch B=4 across 8 cores (2 cores per batch),
sequence-parallel over query rows within a batch (2048 rows per core).
Each core computes K for the full 4096-token sequence and Q for its own
2048 query rows, then its [2048, 4096] slab of the output.

Layouts: host pre-transposes x to [D, M] (fp16) so both MLPs run with the
contraction dim on partitions and produce Q^T/K^T directly; no on-device
transposes are needed. Scores accumulate in fp32 PSUM; a fused custom-DVE
op (TENSOR_MASK_REDUCE) drains PSUM->SBUF while computing the row max;
exp runs on ScalarE with a per-row scale and a fused row-sum; the final
normalize runs on DVE in fp16 (4x mode); the fp16 output slab is upcast
to fp32 on the host.
"""

import numpy as np

import concourse.mybir as mybir
import concourse.tile as tile
from concourse import bacc, bass_utils
from concourse.dve_ops import TENSOR_MASK_REDUCE

# Problem constants (hardcoded per harness contract)
B, M, D_IN, D_HID, D_OUT = 4, 4096, 256, 512, 256
TEMP = 10.0
NCORES = 8
CORES_PER_BATCH = NCORES // B          # 2
MQ = M // CORES_PER_BATCH              # 2048 query rows per core
P = 128                                # partitions
F16 = mybir.dt.float16
F32 = mybir.dt.float32

NEG_INIT = -3.0e38                     # max-reduce init (< any fp32 score)
TB = 1024                              # MLP token block per activation


def _emit_mlp_block(nc, pool_ps, pool_h, w1, w2, xt, yt, t0, pfx):
    """One TB-token block of a two-layer MLP in transposed layout."""
    if True:
        ht = [
            pool_h.tile([P, TB], F16, tag=f"h{m}", name=f"{pfx}h{m}")
            for m in range(D_HID // P)
        ]
        # layer 1: h^T[m] = relu(sum_c W1[c, m*128:+128]^T @ x^T[c])
        for m in range(D_HID // P):
            ph = pool_ps.tile([P, TB], F32, tag="mlp", name="ph")
            for c in range(2):
                for s0 in range(0, TB, 512):
                    nc.tensor.matmul(
                        ph[:, s0 : s0 + 512],
                        w1[c][:, m * P : (m + 1) * P],
                        xt[c][:, t0 + s0 : t0 + s0 + 512],
                        start=(c == 0),
                        stop=(c == 1),
                    )
            nc.scalar.activation(
                out=ht[m],
                in_=ph,
                func=mybir.ActivationFunctionType.Relu,
                bias=0.0,
                scale=1.0,
            )
        # layer 2: y^T[o] = sum_c W2[c, o*128:+128]^T @ h^T[c]
        for o in range(D_OUT // P):
            po = pool_ps.tile([P, TB], F32, tag="mlp", name="po")
            for c in range(4):
                for s0 in range(0, TB, 512):
                    nc.tensor.matmul(
                        po[:, s0 : s0 + 512],
                        w2[c][:, o * P : (o + 1) * P],
                        ht[c][:, s0 : s0 + 512],
                        start=(c == 0),
                        stop=(c == 3),
                    )
            nc.scalar.activation(
                out=yt[o][t0 // TB],
                in_=po,
                func=mybir.ActivationFunctionType.Copy,
                bias=0.0,
                scale=1.0,
            )


def build_nc():
    nc = bacc.Bacc("TRN2", target_bir_lowering=False, debug=False)

    xt_d = nc.dram_tensor("xt", [D_IN, M], F16, kind="ExternalInput")
    xqt_d = nc.dram_tensor("xqt", [D_IN, MQ], F16, kind="ExternalInput")
    w1k_d = nc.dram_tensor("w1k", [D_IN, D_HID], F16, kind="ExternalInput")
    w2k_d = nc.dram_tensor("w2k", [D_HID, D_OUT], F16, kind="ExternalInput")
    w1q_d = nc.dram_tensor("w1q", [D_IN, D_HID], F16, kind="ExternalInput")
    w2q_d = nc.dram_tensor("w2q", [D_HID, D_OUT], F16, kind="ExternalInput")
    out_d = nc.dram_tensor("out", [MQ, M], F16, kind="ExternalOutput")

    with tile.TileContext(nc) as tc:
        with tc.tile_pool(name="const", bufs=1) as cp, \
             tc.tile_pool(name="hpool", bufs=3) as hp, \
             tc.tile_pool(name="spool", bufs=5) as sp, \
             tc.tile_pool(name="epool", bufs=3) as ep, \
             tc.tile_pool(name="small", bufs=22) as smp:

            # ---- load inputs (weights first; x in column chunks) ----
            def load_w(dram, rows, cols, name):
                tiles = []
                for c in range(rows // P):
                    t = cp.tile([P, cols], F16, tag=f"{name}{c}", name=f"{name}{c}")
                    nc.sync.dma_start(out=t, in_=dram.ap()[c * P : (c + 1) * P, :])
                    tiles.append(t)
                return tiles

            w1q = load_w(w1q_d, D_IN, D_HID, "w1q")
            w2q = load_w(w2q_d, D_HID, D_OUT, "w2q")
            w1k = load_w(w1k_d, D_IN, D_HID, "w1k")
            w2k = load_w(w2k_d, D_HID, D_OUT, "w2k")

            xq = [cp.tile([P, MQ], F16, tag=f"xq{c}", name=f"xq{c}") for c in range(2)]
            for t0 in range(0, MQ, TB):
                for c in range(2):
                    nc.scalar.dma_start(
                        out=xq[c][:, t0 : t0 + TB],
                        in_=xqt_d.ap()[c * P : (c + 1) * P, t0 : t0 + TB],
                    )
            xk = [cp.tile([P, M], F16, tag=f"xk{c}", name=f"xk{c}") for c in range(2)]
            for t0 in range(0, M, TB):
                for c in range(2):
                    nc.scalar.dma_start(
                        out=xk[c][:, t0 : t0 + TB],
                        in_=xt_d.ap()[c * P : (c + 1) * P, t0 : t0 + TB],
                    )

            # constants
            c3_q = smp.tile([P, 1], F32, tag="c3q", name="c3q")
            nc.vector.memset(c3_q, float(TB))
            bias_mT = smp.tile([P, 1], F32, tag="bT", name="bT")
            nc.vector.memset(bias_mT, -TEMP)

            # K^T / Q^T in TB-wide segment tiles so downstream S matmuls
            # can start as soon as individual segments are written
            kt = [
                [cp.tile([P, TB], F16, tag=f"kt{c}s{s}", name=f"kt{c}s{s}")
                 for s in range(M // TB)]
                for c in range(2)
            ]
            qt = [
                [cp.tile([P, TB], F16, tag=f"qt{c}s{s}", name=f"qt{c}s{s}")
                 for s in range(MQ // TB)]
                for c in range(2)
            ]

            # ---- MLPs + S phase share one PSUM pool so the scheduler can
            # overlap K-MLP with early S matmuls (segment-level deps).
            # Emission order interleaves K/Q blocks so the first S quarter
            # (kt seg0 + qt seg0) is ready as early as possible. ----
            with tc.tile_pool(name="psum", bufs=2, space="PSUM") as psum:
                for t0 in range(0, MQ, TB):
                    _emit_mlp_block(nc, psum, hp, w1q, w2q, xq, qt, t0, "q")
                for t0 in range(0, M, TB):
                    _emit_mlp_block(nc, psum, hp, w1k, w2k, xk, kt, t0, "k")

                NQ = M // TB                       # 4 quarter-row blocks
                for i in range(MQ // P):           # 16 query tiles
                    s32 = sp.tile([P, M], F32, tag="s32", name="s32")
                    mx4 = smp.tile([P, NQ], F32, tag="mx4", name="mx4")
                    for q in range(NQ):
                        ps = psum.tile(
                            [P, TB], F32,
                            tag="ps" if q < 2 else "mlp", name="ps",
                        )
                        for c in range(2):         # weights outer: fewer loads
                            for n0 in range(0, TB, 512):
                                nc.tensor.matmul(
                                    ps[:, n0 : n0 + 512],
                                    qt[c][i * P // TB][:, i * P % TB : i * P % TB + P],
                                    kt[c][q][:, n0 : n0 + 512],
                                    start=(c == 0),
                                    stop=(c == 1),
                                )
                        # fused PSUM->SBUF copy + row-max
                        nc.vector._custom_dve(
                            TENSOR_MASK_REDUCE,
                            out=s32[:, q * TB : (q + 1) * TB],
                            in0=ps,
                            in1=c3_q,
                            s0=0.0,
                            s1=NEG_INIT,
                            imm2=1.0,
                            accum_out=mx4[:, q : q + 1],
                        )
                    rowmax = smp.tile([P, 1], F32, tag="rowmax", name="rowmax")
                    nc.vector.reduce_max(rowmax, mx4, axis=mybir.AxisListType.X)
                    rmax = smp.tile([P, 1], F32, tag="rmax", name="rmax")
                    nc.vector.reciprocal(rmax, rowmax)
                    scl = smp.tile([P, 1], F32, tag="scl", name="scl")
                    nc.vector.tensor_scalar_mul(scl, rmax, TEMP)
                    rowsum = smp.tile([P, 1], F32, tag="rowsum", name="rowsum")
                    # e = exp(TEMP/rowmax * S - TEMP) -> fp16; rowsum fused fp32
                    e16 = ep.tile([P, M], F16, tag="e16", name="e16")
                    nc.scalar.activation(
                        out=e16,
                        in_=s32,
                        func=mybir.ActivationFunctionType.Exp,
                        bias=bias_mT,
                        scale=scl,
                        accum_out=rowsum,
                    )
                    rsum = smp.tile([P, 1], F32, tag="rsum", name="rsum")
                    nc.vector.reciprocal(rsum, rowsum)
                    # normalize on DVE in fp16 (4x mode), in place
                    nc.vector.tensor_scalar_mul(e16, e16, rsum)
                    nc.sync.dma_start(
                        out=out_d.ap()[i * P : (i + 1) * P, :], in_=e16
                    )

    nc.compile()
    return nc


_NC_CACHE = None


def _get_nc():
    global _NC_CACHE
    if _NC_CACHE is None:
        _NC_CACHE = build_nc()
    return _NC_CACHE


def make_in_maps(x, Wq1, Wq2, Wk1, Wk2):
    """Per-core input dicts (host-side sharding + layout prep)."""
    x = np.asarray(x, dtype=np.float32)
    w1q = np.asarray(Wq1, dtype=np.float16)
    w2q = np.asarray(Wq2, dtype=np.float16)
    w1k = np.asarray(Wk1, dtype=np.float16)
    w2k = np.asarray(Wk2, dtype=np.float16)
    xt16 = [np.ascontiguousarray(x[b].T).astype(np.float16) for b in range(B)]
    in_maps = []
    for c in range(NCORES):
        b, h = divmod(c, CORES_PER_BATCH)
        in_maps.append(
            {
                "xt": xt16[b],
                "xqt": np.ascontiguousarray(xt16[b][:, h * MQ : (h + 1) * MQ]),
                "w1k": w1k,
                "w2k": w2k,
                "w1q": w1q,
                "w2q": w2q,
            }
        )
    return in_maps


def kernel(x, Wq1, bq1, Wq2, bq2, Wk1, bk1, Wk2, bk2):
    for b_ in (bq1, bq2, bk1, bk2):
        assert not np.any(np.asarray(b_)), "kernel assumes zero MLP biases"

    in_maps = make_in_maps(x, Wq1, Wq2, Wk1, Wk2)
    nc = _get_nc()
    res = bass_utils.run_bass_kernel_spmd(nc, in_maps, core_ids=list(range(NCORES)))

    out = np.empty((B, M, M), dtype=np.float32)
    for c, r in enumerate(res.results):
        b, h = divmod(c, CORES_PER_BATCH)
        out[b, h * MQ : (h + 1) * MQ, :] = r["out"].astype(np.float32)
    return out
